# revision 1
# baseline (speedup 1.0000x reference)
"""Trainium2 Bass kernel for nn_DynamicKernelSelection (moe_routing).

Strategy
--------
Host (cheap, O(B*C)):
  * Gating in float64 (argmax margins are far above fp32 noise).
  * Samples are paired by identical (expert1, expert2); at most one
    leftover pair can mix experts (pigeonhole) -- the device then runs that
    pair with slot-0's experts and the slot-1 sample is recomputed on the
    host (fp64, tiny fraction of total work).
  * Depthwise conv -> banded Toeplitz lhsT per (channel, kernel column):
    T[h', h] = W[c, dh, dw] at h' = h + (dh-k//2)*dil.  The H-taps become a
    single fp32 128x128 matmul per kernel column; W-shifts are realized by
    accumulating the k_w matmuls into PSUM at shifted column ranges.

Device (SPMD, 8 cores):
  * Work unit = (pair, channel): both samples of a pair are interleaved in
    the free dim as (w, s) so one N=256 matmul covers the pair.
  * Channels are split 128/8: every core runs 16 channels of EVERY pair,
    so per-pair native kernel sizes (3/5 and 7/9/11) give a uniform
    instruction stream across cores AND perfect load balance.
  * Per unit: k1 matmuls -> PSUM -> bias-add evac (VectorE) -> out1 tile
    (DMAed out, reused as stage-2 rhs) -> k2 matmuls -> PSUM -> evac ->
    out2.  All DMAs are [128-partition x contiguous] transfers.
"""

import numpy as np

B, C, H, W = 16, 128, 128, 128
N_CORES = 8
CPC = C // N_CORES           # channels per core per pair (16)
NPAIR = B // 2               # 8 pairs
DIL1, DIL2 = 1, 3
K1S = {0: 3, 1: 5}           # stage-1 expert -> kernel size
K2S = {0: 7, 1: 9, 2: 11}

_PROGS = {}                  # signature -> compiled program


# --------------------------------------------------------------- host math
def _gating(x, aw1, ab1, aw2, ab2):
    pooled = x.astype(np.float64).mean(axis=(2, 3))
    l1 = pooled @ aw1.astype(np.float64).T + ab1.astype(np.float64)
    l2 = pooled @ aw2.astype(np.float64).T + ab2.astype(np.float64)
    return l1.argmax(axis=1), l2.argmax(axis=1)


def _band(wk, dil):
    """wk: [C, k, k] fp32 -> banded lhsT stack [C, H, k*H] fp32."""
    k = wk.shape[-1]
    t = np.zeros((C, H, k, H), np.float32)
    tv = t.transpose(1, 3, 0, 2)  # [h', h, C, dw] view
    c0 = k // 2
    for dh in range(k):
        d = (dh - c0) * dil
        h = np.arange(max(0, -d), H - max(0, d))
        tv[h + d, h] = wk[:, dh, :]
    return np.ascontiguousarray(t.reshape(C, H, k * H))


def _host_conv(x, wk, b, dil):
    """x [C,H,W] fp64, wk [C,k,k], b [C]: same-padded depthwise conv."""
    k = wk.shape[-1]
    c0 = k // 2
    out = np.zeros_like(x)
    for dh in range(k):
        for dw in range(k):
            dh_, dw_ = (dh - c0) * dil, (dw - c0) * dil
            hs = slice(max(0, -dh_), H - max(0, dh_))
            ws = slice(max(0, -dw_), W - max(0, dw_))
            hs2 = slice(max(0, dh_), H - max(0, -dh_))
            ws2 = slice(max(0, dw_), W - max(0, -dw_))
            out[:, hs, ws] += wk[:, dh, dw][:, None, None] * x[:, hs2, ws2]
    return out + b[:, None, None]


def _pair_samples(idx1, idx2):
    """Pair samples by (e1, e2); leftovers paired preferring same e1.
    Returns pairs [(a, b)] and fixes {sample: 'stage2' | 'both'}."""
    from collections import defaultdict
    groups = defaultdict(list)
    for s in range(B):
        groups[(int(idx1[s]), int(idx2[s]))].append(s)
    pairs, singles = [], []
    for key in sorted(groups):
        lst = groups[key]
        while len(lst) >= 2:
            pairs.append((lst.pop(0), lst.pop(0)))
        if lst:
            singles.append(lst[0])
    # pair leftovers: prefer same e1 (then only stage-2 needs a host fix)
    fixes = {}
    while singles:
        a = singles.pop(0)
        bi = next((i for i, s in enumerate(singles)
                   if idx1[s] == idx1[a]), 0)
        b = singles.pop(bi)
        pairs.append((a, b))
        fixes[b] = "stage2" if idx1[b] == idx1[a] else "both"
    return pairs, fixes


# ------------------------------------------------------------ device program
def _build_program(sig):
    """sig: tuple of (k1, k2) per pair."""
    import concourse.tile as tile
    from concourse import bacc, mybir

    dt = mybir.dt.float32
    f16 = mybir.dt.float16
    add = mybir.AluOpType.add
    sub = mybir.AluOpType.subtract
    nc = bacc.Bacc("TRN2", target_bir_lowering=False, debug=False,
                   enable_asserts=False, num_devices=N_CORES)

    xs_d, t1_d, t2_d, b1_d, b2_d, o1_d, o2_d = [], [], [], [], [], [], []
    for p, (k1, k2) in enumerate(sig):
        xs_d.append(nc.dram_tensor(f"x_{p}", [CPC, H, 2 * W], dt,
                                   kind="ExternalInput").ap())
        t1_d.append(nc.dram_tensor(f"t1_{p}", [CPC, H, k1 * H], dt,
                                   kind="ExternalInput").ap())
        t2_d.append(nc.dram_tensor(f"t2_{p}", [CPC, H, 2 * k2 * H], f16,
                                   kind="ExternalInput").ap())
        b1_d.append(nc.dram_tensor(f"b1_{p}", [H, CPC], dt,
                                   kind="ExternalInput").ap())
        b2_d.append(nc.dram_tensor(f"b2_{p}", [H, CPC], dt,
                                   kind="ExternalInput").ap())
        o1_d.append(nc.dram_tensor(f"o1_{p}", [CPC, H, 2 * W], dt,
                                   kind="ExternalOutput").ap())
        o2_d.append(nc.dram_tensor(f"o2_{p}", [CPC, H, 2 * W], dt,
                                   kind="ExternalOutput").ap())

    def conv_mms(psum, tt, src, k, dil):
        c0 = k // 2
        order = [c0] + [dw for dw in range(k) if dw != c0]
        for j, dw in enumerate(order):
            d = (dw - c0) * dil
            a = max(0, -d)
            ln = W - abs(d)
            nc.tensor.matmul(
                out=psum[:, 2 * a:2 * (a + ln)],
                lhsT=tt[:, dw * H:(dw + 1) * H],
                rhs=src[:, 2 * (a + d):2 * (a + d + ln)],
                start=(j == 0), stop=(j == len(order) - 1),
                skip_group_check=True)

    def conv_mms_f16(pm, pl, tt, srch, srcl, k, dil):
        # fp16 hi/lo 3-pass at true scale: pm += Th@xh; pl += Th@xl + Tl@xh
        c0 = k // 2
        order = [c0] + [dw for dw in range(k) if dw != c0]
        for j, dw in enumerate(order):
            d = (dw - c0) * dil
            a = max(0, -d)
            ln = W - abs(d)
            ocols = slice(2 * a, 2 * (a + ln))
            icols = slice(2 * (a + d), 2 * (a + d + ln))
            th = tt[:, (2 * dw) * H:(2 * dw + 1) * H]
            tl = tt[:, (2 * dw + 1) * H:(2 * dw + 2) * H]
            last = j == len(order) - 1
            nc.tensor.matmul(out=pm[:, ocols], lhsT=th, rhs=srch[:, icols],
                             start=(j == 0), stop=last,
                             skip_group_check=True)
            nc.tensor.matmul(out=pl[:, ocols], lhsT=th, rhs=srcl[:, icols],
                             start=(j == 0), stop=False,
                             skip_group_check=True)
            nc.tensor.matmul(out=pl[:, ocols], lhsT=tl, rhs=srch[:, icols],
                             start=False, stop=last,
                             skip_group_check=True)

    with tile.TileContext(nc) as tc:
        with (tc.tile_pool(name="xp", bufs=6) as xp,
              tc.tile_pool(name="o1p", bufs=4) as o1p,
              tc.tile_pool(name="o2p", bufs=4) as o2p,
              tc.tile_pool(name="t1p", bufs=4) as t1p,
              tc.tile_pool(name="t2p", bufs=4) as t2p,
              tc.tile_pool(name="bp", bufs=2) as bp,
              tc.tile_pool(name="tm", bufs=4) as tm,
              tc.tile_pool(name="ps", bufs=8, space="PSUM") as ps):
            # software-pipelined two units deep: PE never waits on the
            # DVE evac + hi/lo split chain of the previous units.
            pend = []

            def emit_stage2(st):
                p, u, k2, o1h, o1l, b2t = st
                t2t = t2p.tile([128, 2 * k2 * H], f16, tag="t2")
                nc.sync.dma_start(out=t2t[:], in_=t2_d[p][u])
                pm2 = ps.tile([128, 2 * W], dt, tag="ps")
                pl2 = ps.tile([128, 2 * W], dt, tag="ps")
                conv_mms_f16(pm2, pl2, t2t, o1h, o1l, k2, DIL2)
                o2c = o2p.tile([128, 2 * W], dt, tag="o2")
                tmp = tm.tile([128, 2 * W], dt, tag="tmp")
                nc.vector.tensor_copy(out=tmp[:], in_=pl2[:])
                nc.vector.scalar_tensor_tensor(
                    out=o2c[:], in0=pm2[:], scalar=b2t[:, u:u + 1],
                    in1=tmp[:], op0=add, op1=add)
                nc.sync.dma_start(out=o2_d[p][u], in_=o2c[:])

            for p, (k1, k2) in enumerate(sig):
                b1t = bp.tile([128, CPC], dt, tag="b1")
                nc.sync.dma_start(out=b1t[:], in_=b1_d[p])
                b2t = bp.tile([128, CPC], dt, tag="b2")
                nc.sync.dma_start(out=b2t[:], in_=b2_d[p])
                for u in range(CPC):
                    xc = xp.tile([128, 2 * W], dt, tag="x")
                    nc.sync.dma_start(out=xc[:], in_=xs_d[p][u])
                    t1t = t1p.tile([128, k1 * H], dt, tag="t1")
                    nc.sync.dma_start(out=t1t[:], in_=t1_d[p][u])
                    p1 = ps.tile([128, 2 * W], dt, tag="ps")
                    conv_mms(p1, t1t, xc, k1, DIL1)
                    o1c = o1p.tile([128, 2 * W], dt, tag="o1")
                    nc.vector.tensor_scalar(out=o1c[:], in0=p1[:],
                                            scalar1=b1t[:, u:u + 1],
                                            scalar2=None, op0=add)
                    nc.sync.dma_start(out=o1_d[p][u], in_=o1c[:])
                    # split o1 into fp16 hi + lo for the fp16 stage 2
                    o1h = o1p.tile([128, 2 * W], f16, tag="o1h")
                    nc.vector.tensor_copy(out=o1h[:], in_=o1c[:])
                    o1l = o1p.tile([128, 2 * W], f16, tag="o1l")
                    nc.vector.tensor_tensor(out=o1l[:], in0=o1c[:],
                                            in1=o1h[:], op=sub)
                    pend.append((p, u, k2, o1h, o1l, b2t))
                    if len(pend) > 3:
                        emit_stage2(pend.pop(0))
            while pend:
                emit_stage2(pend.pop(0))
    nc.compile()
    return nc


# ------------------------------------------------------------------- driver
def kernel(x, aw1, ab1, aw2, ab2, w1_3, b1_3, w1_5, b1_5,
           w2_7, b2_7, w2_9, b2_9, w2_11, b2_11):
    from concourse.bass_utils import run_bass_kernel_spmd

    x = np.ascontiguousarray(np.asarray(x, dtype=np.float32))
    assert x.shape == (B, C, H, W)

    idx1, idx2 = _gating(np.asarray(x), np.asarray(aw1), np.asarray(ab1),
                         np.asarray(aw2), np.asarray(ab2))
    pairs, fixes = _pair_samples(idx1, idx2)

    w1e = [np.ascontiguousarray(np.asarray(w, np.float32)[:, 0])
           for w in (w1_3, w1_5)]
    w2e = [np.ascontiguousarray(np.asarray(w, np.float32)[:, 0])
           for w in (w2_7, w2_9, w2_11)]
    b1e = [np.asarray(b, np.float32) for b in (b1_3, b1_5)]
    b2e = [np.asarray(b, np.float32) for b in (b2_7, b2_9, b2_11)]

    # per-pair experts = slot-0's selection
    pe1 = [int(idx1[a]) for a, _ in pairs]
    pe2 = [int(idx2[a]) for a, _ in pairs]
    sig = tuple((K1S[e1], K2S[e2]) for e1, e2 in zip(pe1, pe2))

    if sig not in _PROGS:
        _PROGS[sig] = _build_program(sig)
    nc = _PROGS[sig]

    def _band_hilo(wk, dil):
        k = wk.shape[-1]
        band = _band(wk, dil).reshape(C, H, k, H)
        hi = band.astype(np.float16)
        lo = (band - hi.astype(np.float32)).astype(np.float16)
        return np.ascontiguousarray(
            np.stack([hi, lo], axis=3).reshape(C, H, 2 * k * H))

    # stage-1 lhsT fp32; stage-2 lhsT fp16 hi/lo per distinct expert
    t1b = {e: _band(w1e[e], DIL1) for e in set(pe1)}
    t2b = {e: _band_hilo(w2e[e], DIL2) for e in set(pe2)}

    # per-pair interleaved x: [C, H, W, 2] -> [C, H, 2W]
    xpair = []
    for a, b in pairs:
        xi = np.stack([x[a], x[b]], axis=-1).reshape(C, H, 2 * W)
        xpair.append(xi)

    in_maps = []
    for core in range(N_CORES):
        cs = slice(core * CPC, (core + 1) * CPC)
        m = {}
        for p, (e1, e2) in enumerate(zip(pe1, pe2)):
            m[f"x_{p}"] = xpair[p][cs]
            m[f"t1_{p}"] = t1b[e1][cs]
            m[f"t2_{p}"] = t2b[e2][cs]
            m[f"b1_{p}"] = np.ascontiguousarray(
                np.broadcast_to(b1e[e1][None, cs], (H, CPC)))
            m[f"b2_{p}"] = np.ascontiguousarray(
                np.broadcast_to(b2e[e2][None, cs], (H, CPC)))
        in_maps.append(m)

    res = run_bass_kernel_spmd(nc, in_maps, list(range(N_CORES)))

    out1 = np.empty((B, C, H, W), np.float32)
    out2 = np.empty((B, C, H, W), np.float32)
    for core in range(N_CORES):
        cs = slice(core * CPC, (core + 1) * CPC)
        r = res.results[core]
        for p, (a, b) in enumerate(pairs):
            o1 = r[f"o1_{p}"].reshape(CPC, H, W, 2)
            o2 = r[f"o2_{p}"].reshape(CPC, H, W, 2)
            out1[a, cs], out1[b, cs] = o1[..., 0], o1[..., 1]
            out2[a, cs], out2[b, cs] = o2[..., 0], o2[..., 1]

    # host fix-up for mixed pairs (at most 2 samples)
    for s, kind in fixes.items():
        e1, e2 = int(idx1[s]), int(idx2[s])
        if kind == "both":
            o1 = _host_conv(x[s].astype(np.float64), w1e[e1].astype(np.float64),
                            b1e[e1].astype(np.float64), DIL1)
            out1[s] = o1.astype(np.float32)
        else:
            o1 = out1[s].astype(np.float64)
        out2[s] = _host_conv(o1, w2e[e2].astype(np.float64),
                             b2e[e2].astype(np.float64), DIL2).astype(np.float32)
    return out1, out2



# revision 4
# speedup vs baseline: 3.1367x; 3.1367x over previous
"""Trainium2 Bass kernel for nn_DynamicKernelSelection (moe_routing).

Strategy
--------
Host (cheap, O(B*C)):
  * Gating in float64 (argmax margins are far above fp32 noise).
  * Samples are paired by identical (expert1, expert2); at most one
    leftover pair can mix experts (pigeonhole) -- the device then runs that
    pair with slot-0's experts and the slot-1 sample is recomputed on the
    host (fp64, tiny fraction of total work).
  * Depthwise conv -> banded Toeplitz lhsT per (channel, kernel column):
    T[h', h] = W[c, dh, dw] at h' = h + (dh-k//2)*dil.  The H-taps become a
    single 128x128 matmul per kernel column; W-shifts are realized by
    accumulating the k_w matmuls into PSUM at shifted column ranges.

Device (SPMD, 8 cores; all fp16 single-pass -- tolerance is 2e-2, fp16
single-pass error is ~1e-3):
  * Work unit = (channel, pair): both samples of a pair are interleaved in
    the free dim as (w, s) so one N=256 matmul covers the pair.
  * Channels split 128/8 across cores; every core runs 8 pairs x 16
    channels -> uniform instruction stream and perfect load balance.
  * Channel-major loop: per channel, ONE x DMA (all pairs), ONE Toeplitz
    DMA (all experts, reused by all pairs -- 8x less traffic than
    per-pair), then k1 matmuls -> PSUM -> bias evac to fp16 (VectorE)
    per pair, stage-2 k2 matmuls on the fp16 out1 tile -> evac -> out2.
  * out1/out2 leave the device as fp16 (halves write traffic); the host
    upcasts to fp32.
"""

import numpy as np

B, C, H, W = 16, 128, 128, 128
N_CORES = 8
CPC = C // N_CORES           # channels per core (16)
NPAIR = B // 2               # 8 pairs
DIL1, DIL2 = 1, 3
K1S = {0: 3, 1: 5}           # stage-1 expert -> kernel size
K2S = {0: 7, 1: 9, 2: 11}

_PROGS = {}                  # signature -> compiled program


# --------------------------------------------------------------- host math
def _gating(x, aw1, ab1, aw2, ab2):
    pooled = x.astype(np.float64).mean(axis=(2, 3))
    l1 = pooled @ aw1.astype(np.float64).T + ab1.astype(np.float64)
    l2 = pooled @ aw2.astype(np.float64).T + ab2.astype(np.float64)
    return l1.argmax(axis=1), l2.argmax(axis=1)


def _band(wk, dil):
    """wk: [C, k, k] fp32 -> banded lhsT stack [C, H, k*H] fp16."""
    k = wk.shape[-1]
    t = np.zeros((C, H, k, H), np.float32)
    tv = t.transpose(1, 3, 0, 2)  # [h', h, C, dw] view
    c0 = k // 2
    for dh in range(k):
        d = (dh - c0) * dil
        h = np.arange(max(0, -d), H - max(0, d))
        tv[h + d, h] = wk[:, dh, :]
    return np.ascontiguousarray(t.reshape(C, H, k * H).astype(np.float16))


def _host_conv(x, wk, b, dil):
    """x [C,H,W] fp64, wk [C,k,k], b [C]: same-padded depthwise conv."""
    k = wk.shape[-1]
    c0 = k // 2
    out = np.zeros_like(x)
    for dh in range(k):
        for dw in range(k):
            dh_, dw_ = (dh - c0) * dil, (dw - c0) * dil
            hs = slice(max(0, -dh_), H - max(0, dh_))
            ws = slice(max(0, -dw_), W - max(0, dw_))
            hs2 = slice(max(0, dh_), H - max(0, -dh_))
            ws2 = slice(max(0, dw_), W - max(0, -dw_))
            out[:, hs, ws] += wk[:, dh, dw][:, None, None] * x[:, hs2, ws2]
    return out + b[:, None, None]


def _pair_samples(idx1, idx2):
    """Pair samples by (e1, e2); leftovers paired preferring same e1.
    Returns pairs [(a, b)] and fixes {sample: 'stage2' | 'both'}."""
    from collections import defaultdict
    groups = defaultdict(list)
    for s in range(B):
        groups[(int(idx1[s]), int(idx2[s]))].append(s)
    pairs, singles = [], []
    for key in sorted(groups):
        lst = groups[key]
        while len(lst) >= 2:
            pairs.append((lst.pop(0), lst.pop(0)))
        if lst:
            singles.append(lst[0])
    fixes = {}
    while singles:
        a = singles.pop(0)
        bi = next((i for i, s in enumerate(singles)
                   if idx1[s] == idx1[a]), 0)
        b = singles.pop(bi)
        pairs.append((a, b))
        fixes[b] = "stage2" if idx1[b] == idx1[a] else "both"
    return pairs, fixes


# ------------------------------------------------------------ device program
def _build_program(sig):
    """sig: (tuple of (k1, k2) per pair, tuple of stage-1 ks, stage-2 ks).

    The Toeplitz pack per channel is laid out as the stage-1 expert
    matrices (in e1ks order) followed by stage-2 (e2ks order)."""
    import concourse.tile as tile
    from concourse import bacc, mybir

    pair_ks, e1ks, e2ks = sig
    dt = mybir.dt.float32
    f16 = mybir.dt.float16
    add = mybir.AluOpType.add
    nc = bacc.Bacc("TRN2", target_bir_lowering=False, debug=False,
                   enable_asserts=False, num_devices=N_CORES)

    # Toeplitz column offsets (in elements) per kernel size, pack order
    toff = {}
    off = 0
    for k in e1ks:
        toff[(1, k)] = off
        off += k * H
    for k in e2ks:
        toff[(2, k)] = off
        off += k * H
    TOT = off

    NW = NPAIR * 2 * W  # free-dim elements per channel (2048)
    x_d = nc.dram_tensor("x", [CPC, H, NW], f16, kind="ExternalInput").ap()
    t_d = nc.dram_tensor("t", [CPC, H, TOT], f16, kind="ExternalInput").ap()
    b_d = nc.dram_tensor("b", [H, CPC * NPAIR * 2], dt,
                         kind="ExternalInput").ap()
    o1_d = nc.dram_tensor("o1", [CPC, H, NW], f16, kind="ExternalOutput").ap()
    o2_d = nc.dram_tensor("o2", [CPC, H, NW], f16, kind="ExternalOutput").ap()

    def conv_mms(psum, tt, tbase, src, sbase, k, dil):
        c0 = k // 2
        order = [c0] + [dw for dw in range(k) if dw != c0]
        for j, dw in enumerate(order):
            d = (dw - c0) * dil
            a = max(0, -d)
            ln = W - abs(d)
            nc.tensor.matmul(
                out=psum[:, 2 * a:2 * (a + ln)],
                lhsT=tt[:, tbase + dw * H:tbase + (dw + 1) * H],
                rhs=src[:, sbase + 2 * (a + d):sbase + 2 * (a + d + ln)],
                start=(j == 0), stop=(j == len(order) - 1),
                skip_group_check=True)

    with tile.TileContext(nc) as tc:
        with (tc.tile_pool(name="xp", bufs=3) as xp,
              tc.tile_pool(name="tp", bufs=3) as tp,
              tc.tile_pool(name="o1p", bufs=3) as o1p,
              tc.tile_pool(name="o2p", bufs=3) as o2p,
              tc.tile_pool(name="bp", bufs=1) as bp,
              tc.tile_pool(name="ps", bufs=8, space="PSUM") as ps):
            bt = bp.tile([128, CPC * NPAIR * 2], dt, tag="b")
            nc.sync.dma_start(out=bt[:], in_=b_d)
            for u in range(CPC):
                xt = xp.tile([128, NW], f16, tag="x")
                nc.sync.dma_start(out=xt[:], in_=x_d[u])
                tt = tp.tile([128, TOT], f16, tag="t")
                nc.sync.dma_start(out=tt[:], in_=t_d[u])
                o1t = o1p.tile([128, NW], f16, tag="o1")
                o2t = o2p.tile([128, NW], f16, tag="o2")
                for p, (k1, k2) in enumerate(pair_ks):
                    cols = slice(p * 2 * W, (p + 1) * 2 * W)
                    p1 = ps.tile([128, 2 * W], dt, tag="ps")
                    conv_mms(p1, tt, toff[(1, k1)], xt, p * 2 * W, k1, DIL1)
                    nc.vector.tensor_scalar(
                        out=o1t[:, cols], in0=p1[:],
                        scalar1=bt[:, (u * NPAIR + p) * 2:(u * NPAIR + p) * 2 + 1],
                        scalar2=None, op0=add)
                for p, (k1, k2) in enumerate(pair_ks):
                    cols = slice(p * 2 * W, (p + 1) * 2 * W)
                    p2 = ps.tile([128, 2 * W], dt, tag="ps")
                    conv_mms(p2, tt, toff[(2, k2)], o1t, p * 2 * W, k2, DIL2)
                    nc.vector.tensor_scalar(
                        out=o2t[:, cols], in0=p2[:],
                        scalar1=bt[:, (u * NPAIR + p) * 2 + 1:(u * NPAIR + p) * 2 + 2],
                        scalar2=None, op0=add)
                nc.sync.dma_start(out=o1_d[u], in_=o1t[:])
                nc.sync.dma_start(out=o2_d[u], in_=o2t[:])
    nc.compile()
    return nc


# ------------------------------------------------------------------- driver
def kernel(x, aw1, ab1, aw2, ab2, w1_3, b1_3, w1_5, b1_5,
           w2_7, b2_7, w2_9, b2_9, w2_11, b2_11):
    from concourse.bass_utils import run_bass_kernel_spmd

    x = np.ascontiguousarray(np.asarray(x, dtype=np.float32))
    assert x.shape == (B, C, H, W)

    idx1, idx2 = _gating(np.asarray(x), np.asarray(aw1), np.asarray(ab1),
                         np.asarray(aw2), np.asarray(ab2))
    pairs, fixes = _pair_samples(idx1, idx2)

    w1e = [np.ascontiguousarray(np.asarray(w, np.float32)[:, 0])
           for w in (w1_3, w1_5)]
    w2e = [np.ascontiguousarray(np.asarray(w, np.float32)[:, 0])
           for w in (w2_7, w2_9, w2_11)]
    b1e = [np.asarray(b, np.float32) for b in (b1_3, b1_5)]
    b2e = [np.asarray(b, np.float32) for b in (b2_7, b2_9, b2_11)]

    # per-pair experts = slot-0's selection
    pe1 = [int(idx1[a]) for a, _ in pairs]
    pe2 = [int(idx2[a]) for a, _ in pairs]
    e1ks = tuple(sorted({K1S[e] for e in pe1}))
    e2ks = tuple(sorted({K2S[e] for e in pe2}))
    pair_ks = tuple((K1S[e1], K2S[e2]) for e1, e2 in zip(pe1, pe2))
    sig = (pair_ks, e1ks, e2ks)

    if sig not in _PROGS:
        _PROGS[sig] = _build_program(sig)
    nc = _PROGS[sig]

    # fp16 banded lhsT per distinct expert, packed [C, H, TOT]
    packs = []
    for e in sorted({e for e in pe1}, key=lambda e: K1S[e]):
        packs.append(_band(w1e[e], DIL1))
    for e in sorted({e for e in pe2}, key=lambda e: K2S[e]):
        packs.append(_band(w2e[e], DIL2))
    tpack = np.concatenate(packs, axis=2)  # [C, H, TOT]

    # x packed [C, H, NPAIR*2W] fp16: per pair interleaved (w, s)
    ab = np.array(pairs)                     # [NPAIR, 2]
    xsel = x[ab.reshape(-1)].reshape(NPAIR, 2, C, H, W)
    xpk = np.ascontiguousarray(
        xsel.transpose(2, 3, 0, 4, 1), dtype=np.float16).reshape(C, H, -1)

    # biases [H, CPC*NPAIR*2] per core (broadcast along H)
    bsel = np.empty((CPC, NPAIR, 2), np.float32)

    in_maps = []
    for core in range(N_CORES):
        cs = slice(core * CPC, (core + 1) * CPC)
        for ui, c in enumerate(range(core * CPC, (core + 1) * CPC)):
            for p, (e1, e2) in enumerate(zip(pe1, pe2)):
                bsel[ui, p, 0] = b1e[e1][c]
                bsel[ui, p, 1] = b2e[e2][c]
        m = {"x": xpk[cs], "t": tpack[cs],
             "b": np.ascontiguousarray(
                 np.broadcast_to(bsel.reshape(1, -1), (H, CPC * NPAIR * 2)))}
        in_maps.append(m)

    res = run_bass_kernel_spmd(nc, in_maps, list(range(N_CORES)))

    out1 = np.empty((B, C, H, W), np.float32)
    out2 = np.empty((B, C, H, W), np.float32)
    for core in range(N_CORES):
        cs = slice(core * CPC, (core + 1) * CPC)
        r = res.results[core]
        o1 = r["o1"].reshape(CPC, H, NPAIR, W, 2).astype(np.float32)
        o2 = r["o2"].reshape(CPC, H, NPAIR, W, 2).astype(np.float32)
        for p, (a, b) in enumerate(pairs):
            out1[a, cs] = o1[:, :, p, :, 0]
            out1[b, cs] = o1[:, :, p, :, 1]
            out2[a, cs] = o2[:, :, p, :, 0]
            out2[b, cs] = o2[:, :, p, :, 1]

    # host fix-up for mixed pairs (at most 2 samples)
    for s, kind in fixes.items():
        e1, e2 = int(idx1[s]), int(idx2[s])
        if kind == "both":
            o1 = _host_conv(x[s].astype(np.float64), w1e[e1].astype(np.float64),
                            b1e[e1].astype(np.float64), DIL1)
            out1[s] = o1.astype(np.float32)
        else:
            o1 = out1[s].astype(np.float64)
        out2[s] = _host_conv(o1, w2e[e2].astype(np.float64),
                             b2e[e2].astype(np.float64), DIL2).astype(np.float32)
    return out1, out2


# revision 6
# speedup vs baseline: 3.2103x; 1.0235x over previous
"""Trainium2 Bass kernel for nn_DynamicKernelSelection (moe_routing).

Strategy
--------
Host (cheap, O(B*C)):
  * Gating in float64 (argmax margins are far above fp32 noise).
  * Samples are paired by identical (expert1, expert2); at most one
    leftover pair can mix experts (pigeonhole) -- the device then runs that
    pair with slot-0's experts and the slot-1 sample is recomputed on the
    host (fp64, tiny fraction of total work).
  * Depthwise conv -> banded Toeplitz lhsT per (channel, kernel column):
    T[h', h] = W[c, dh, dw] at h' = h + (dh-k//2)*dil.  The H-taps become a
    single 128x128 matmul per kernel column; W-shifts are realized by
    accumulating the k_w matmuls into PSUM at shifted column ranges.

Device (SPMD, 8 cores; all fp16 single-pass -- tolerance is 2e-2, fp16
single-pass error is ~1e-3):
  * Work unit = (channel, pair): both samples of a pair are interleaved in
    the free dim as (w, s) so one N=256 matmul covers the pair.
  * Channels split 128/8 across cores; every core runs 8 pairs x 16
    channels -> uniform instruction stream and perfect load balance.
  * Channel-major loop: per channel, ONE x DMA (all pairs), ONE Toeplitz
    DMA (all experts, reused by all pairs -- 8x less traffic than
    per-pair), then k1 matmuls -> PSUM -> bias evac to fp16 (VectorE)
    per pair, stage-2 k2 matmuls on the fp16 out1 tile -> evac -> out2.
  * out1/out2 leave the device as fp16 (halves write traffic); the host
    upcasts to fp32.
"""

import numpy as np

B, C, H, W = 16, 128, 128, 128
N_CORES = 8
CPC = C // N_CORES           # channels per core (16)
NPAIR = B // 2               # 8 pairs
DIL1, DIL2 = 1, 3
K1S = {0: 3, 1: 5}           # stage-1 expert -> kernel size
K2S = {0: 7, 1: 9, 2: 11}

_PROGS = {}                  # signature -> compiled program


# --------------------------------------------------------------- host math
def _gating(x, aw1, ab1, aw2, ab2):
    pooled = x.astype(np.float64).mean(axis=(2, 3))
    l1 = pooled @ aw1.astype(np.float64).T + ab1.astype(np.float64)
    l2 = pooled @ aw2.astype(np.float64).T + ab2.astype(np.float64)
    return l1.argmax(axis=1), l2.argmax(axis=1)


def _band(wk, dil):
    """wk: [C, k, k] fp32 -> banded lhsT stack [C, H, k*H] fp16."""
    k = wk.shape[-1]
    t = np.zeros((C, H, k, H), np.float32)
    tv = t.transpose(1, 3, 0, 2)  # [h', h, C, dw] view
    c0 = k // 2
    for dh in range(k):
        d = (dh - c0) * dil
        h = np.arange(max(0, -d), H - max(0, d))
        tv[h + d, h] = wk[:, dh, :]
    return np.ascontiguousarray(t.reshape(C, H, k * H).astype(np.float16))


def _host_conv(x, wk, b, dil):
    """x [C,H,W] fp64, wk [C,k,k], b [C]: same-padded depthwise conv."""
    k = wk.shape[-1]
    c0 = k // 2
    out = np.zeros_like(x)
    for dh in range(k):
        for dw in range(k):
            dh_, dw_ = (dh - c0) * dil, (dw - c0) * dil
            hs = slice(max(0, -dh_), H - max(0, dh_))
            ws = slice(max(0, -dw_), W - max(0, dw_))
            hs2 = slice(max(0, dh_), H - max(0, -dh_))
            ws2 = slice(max(0, dw_), W - max(0, -dw_))
            out[:, hs, ws] += wk[:, dh, dw][:, None, None] * x[:, hs2, ws2]
    return out + b[:, None, None]


def _pair_samples(idx1, idx2):
    """Pair samples by (e1, e2); leftovers paired preferring same e1.
    Returns pairs [(a, b)] and fixes {sample: 'stage2' | 'both'}."""
    from collections import defaultdict
    groups = defaultdict(list)
    for s in range(B):
        groups[(int(idx1[s]), int(idx2[s]))].append(s)
    pairs, singles = [], []
    for key in sorted(groups):
        lst = groups[key]
        while len(lst) >= 2:
            pairs.append((lst.pop(0), lst.pop(0)))
        if lst:
            singles.append(lst[0])
    fixes = {}
    while singles:
        a = singles.pop(0)
        bi = next((i for i, s in enumerate(singles)
                   if idx1[s] == idx1[a]), 0)
        b = singles.pop(bi)
        pairs.append((a, b))
        fixes[b] = "stage2" if idx1[b] == idx1[a] else "both"
    return pairs, fixes


# ------------------------------------------------------------ device program
def _build_program(sig):
    """sig: (tuple of (k1, k2) per pair, tuple of stage-1 ks, stage-2 ks).

    The Toeplitz pack per channel is laid out as the stage-1 expert
    matrices (in e1ks order) followed by stage-2 (e2ks order)."""
    import concourse.tile as tile
    from concourse import bacc, mybir

    pair_ks, e1ks, e2ks = sig
    dt = mybir.dt.float32
    f16 = mybir.dt.float16
    add = mybir.AluOpType.add
    nc = bacc.Bacc("TRN2", target_bir_lowering=False, debug=False,
                   enable_asserts=False, num_devices=N_CORES)

    # Toeplitz column offsets (in elements) per kernel size, pack order
    toff = {}
    off = 0
    for k in e1ks:
        toff[(1, k)] = off
        off += k * H
    for k in e2ks:
        toff[(2, k)] = off
        off += k * H
    TOT = off

    NW = NPAIR * 2 * W  # free-dim elements per channel (2048)
    x_d = nc.dram_tensor("x", [CPC, H, NW], f16, kind="ExternalInput").ap()
    t_d = nc.dram_tensor("t", [CPC, H, TOT], f16, kind="ExternalInput").ap()
    b_d = nc.dram_tensor("b", [H, CPC * NPAIR * 2], dt,
                         kind="ExternalInput").ap()
    o1_d = nc.dram_tensor("o1", [CPC, H, NW], f16, kind="ExternalOutput").ap()
    o2_d = nc.dram_tensor("o2", [CPC, H, NW], f16, kind="ExternalOutput").ap()

    def conv_mms(psum, tt, tbase, src, sbase, k, dil):
        c0 = k // 2
        order = [c0] + [dw for dw in range(k) if dw != c0]
        for j, dw in enumerate(order):
            d = (dw - c0) * dil
            a = max(0, -d)
            ln = W - abs(d)
            nc.tensor.matmul(
                out=psum[:, 2 * a:2 * (a + ln)],
                lhsT=tt[:, tbase + dw * H:tbase + (dw + 1) * H],
                rhs=src[:, sbase + 2 * (a + d):sbase + 2 * (a + d + ln)],
                start=(j == 0), stop=(j == len(order) - 1),
                skip_group_check=True)

    with tile.TileContext(nc) as tc:
        with (tc.tile_pool(name="xp", bufs=3) as xp,
              tc.tile_pool(name="tp", bufs=3) as tp,
              tc.tile_pool(name="o1p", bufs=3) as o1p,
              tc.tile_pool(name="o2p", bufs=3) as o2p,
              tc.tile_pool(name="bp", bufs=1) as bp,
              tc.tile_pool(name="ps", bufs=8, space="PSUM") as ps):
            # stage-1 vs stage-2 halves of the Toeplitz pack (split the DMA
            # so the first matmuls' dependencies land first)
            T1C = sum(k * H for k in e1ks)
            bt = None
            for u in range(CPC):
                xt = xp.tile([128, NW], f16, tag="x")
                nc.sync.dma_start(out=xt[:], in_=x_d[u])
                tt = tp.tile([128, TOT], f16, tag="t")
                nc.sync.dma_start(out=tt[:, 0:T1C], in_=t_d[u][:, 0:T1C])
                nc.sync.dma_start(out=tt[:, T1C:TOT], in_=t_d[u][:, T1C:TOT])
                if bt is None:
                    bt = bp.tile([128, CPC * NPAIR * 2], dt, tag="b")
                    nc.sync.dma_start(out=bt[:], in_=b_d)
                o1t = o1p.tile([128, NW], f16, tag="o1")
                o2t = o2p.tile([128, NW], f16, tag="o2")
                HP = NPAIR // 2
                for p, (k1, k2) in enumerate(pair_ks):
                    cols = slice(p * 2 * W, (p + 1) * 2 * W)
                    p1 = ps.tile([128, 2 * W], dt, tag="ps")
                    conv_mms(p1, tt, toff[(1, k1)], xt, p * 2 * W, k1, DIL1)
                    nc.vector.tensor_scalar(
                        out=o1t[:, cols], in0=p1[:],
                        scalar1=bt[:, (u * NPAIR + p) * 2:(u * NPAIR + p) * 2 + 1],
                        scalar2=None, op0=add)
                    if p == HP - 1:
                        nc.sync.dma_start(out=o1_d[u][:, :HP * 2 * W],
                                          in_=o1t[:, :HP * 2 * W])
                nc.sync.dma_start(out=o1_d[u][:, HP * 2 * W:],
                                  in_=o1t[:, HP * 2 * W:])
                for p, (k1, k2) in enumerate(pair_ks):
                    cols = slice(p * 2 * W, (p + 1) * 2 * W)
                    p2 = ps.tile([128, 2 * W], dt, tag="ps")
                    conv_mms(p2, tt, toff[(2, k2)], o1t, p * 2 * W, k2, DIL2)
                    nc.vector.tensor_scalar(
                        out=o2t[:, cols], in0=p2[:],
                        scalar1=bt[:, (u * NPAIR + p) * 2 + 1:(u * NPAIR + p) * 2 + 2],
                        scalar2=None, op0=add)
                    if p == HP - 1:
                        nc.sync.dma_start(out=o2_d[u][:, :HP * 2 * W],
                                          in_=o2t[:, :HP * 2 * W])
                nc.sync.dma_start(out=o2_d[u][:, HP * 2 * W:],
                                  in_=o2t[:, HP * 2 * W:])
    nc.compile()
    return nc


# ------------------------------------------------------------------- driver
def kernel(x, aw1, ab1, aw2, ab2, w1_3, b1_3, w1_5, b1_5,
           w2_7, b2_7, w2_9, b2_9, w2_11, b2_11):
    from concourse.bass_utils import run_bass_kernel_spmd

    x = np.ascontiguousarray(np.asarray(x, dtype=np.float32))
    assert x.shape == (B, C, H, W)

    idx1, idx2 = _gating(np.asarray(x), np.asarray(aw1), np.asarray(ab1),
                         np.asarray(aw2), np.asarray(ab2))
    pairs, fixes = _pair_samples(idx1, idx2)

    w1e = [np.ascontiguousarray(np.asarray(w, np.float32)[:, 0])
           for w in (w1_3, w1_5)]
    w2e = [np.ascontiguousarray(np.asarray(w, np.float32)[:, 0])
           for w in (w2_7, w2_9, w2_11)]
    b1e = [np.asarray(b, np.float32) for b in (b1_3, b1_5)]
    b2e = [np.asarray(b, np.float32) for b in (b2_7, b2_9, b2_11)]

    # per-pair experts = slot-0's selection
    pe1 = [int(idx1[a]) for a, _ in pairs]
    pe2 = [int(idx2[a]) for a, _ in pairs]
    e1ks = tuple(sorted({K1S[e] for e in pe1}))
    e2ks = tuple(sorted({K2S[e] for e in pe2}))
    pair_ks = tuple((K1S[e1], K2S[e2]) for e1, e2 in zip(pe1, pe2))
    sig = (pair_ks, e1ks, e2ks)

    if sig not in _PROGS:
        _PROGS[sig] = _build_program(sig)
    nc = _PROGS[sig]

    # fp16 banded lhsT per distinct expert, packed [C, H, TOT]
    packs = []
    for e in sorted({e for e in pe1}, key=lambda e: K1S[e]):
        packs.append(_band(w1e[e], DIL1))
    for e in sorted({e for e in pe2}, key=lambda e: K2S[e]):
        packs.append(_band(w2e[e], DIL2))
    tpack = np.concatenate(packs, axis=2)  # [C, H, TOT]

    # x packed [C, H, NPAIR*2W] fp16: per pair interleaved (w, s)
    ab = np.array(pairs)                     # [NPAIR, 2]
    xsel = x[ab.reshape(-1)].reshape(NPAIR, 2, C, H, W)
    xpk = np.ascontiguousarray(
        xsel.transpose(2, 3, 0, 4, 1), dtype=np.float16).reshape(C, H, -1)

    # biases [H, CPC*NPAIR*2] per core (broadcast along H)
    bsel = np.empty((CPC, NPAIR, 2), np.float32)

    in_maps = []
    for core in range(N_CORES):
        cs = slice(core * CPC, (core + 1) * CPC)
        for ui, c in enumerate(range(core * CPC, (core + 1) * CPC)):
            for p, (e1, e2) in enumerate(zip(pe1, pe2)):
                bsel[ui, p, 0] = b1e[e1][c]
                bsel[ui, p, 1] = b2e[e2][c]
        m = {"x": xpk[cs], "t": tpack[cs],
             "b": np.ascontiguousarray(
                 np.broadcast_to(bsel.reshape(1, -1), (H, CPC * NPAIR * 2)))}
        in_maps.append(m)

    res = run_bass_kernel_spmd(nc, in_maps, list(range(N_CORES)))

    out1 = np.empty((B, C, H, W), np.float32)
    out2 = np.empty((B, C, H, W), np.float32)
    for core in range(N_CORES):
        cs = slice(core * CPC, (core + 1) * CPC)
        r = res.results[core]
        o1 = r["o1"].reshape(CPC, H, NPAIR, W, 2).astype(np.float32)
        o2 = r["o2"].reshape(CPC, H, NPAIR, W, 2).astype(np.float32)
        for p, (a, b) in enumerate(pairs):
            out1[a, cs] = o1[:, :, p, :, 0]
            out1[b, cs] = o1[:, :, p, :, 1]
            out2[a, cs] = o2[:, :, p, :, 0]
            out2[b, cs] = o2[:, :, p, :, 1]

    # host fix-up for mixed pairs (at most 2 samples)
    for s, kind in fixes.items():
        e1, e2 = int(idx1[s]), int(idx2[s])
        if kind == "both":
            o1 = _host_conv(x[s].astype(np.float64), w1e[e1].astype(np.float64),
                            b1e[e1].astype(np.float64), DIL1)
            out1[s] = o1.astype(np.float32)
        else:
            o1 = out1[s].astype(np.float64)
        out2[s] = _host_conv(o1, w2e[e2].astype(np.float64),
                             b2e[e2].astype(np.float64), DIL2).astype(np.float32)
    return out1, out2
